# revision 5
# baseline (speedup 1.0000x reference)
"""HGNN (2-stage hypergraph conv) kernel for Trainium2.

Data-parallel over batch across 8 NeuronCores (16 batches/core). All heavy
matmuls run on the PE in float32r (full-rate fp32-reduced mode, ~1.6e-4/op).

Per-core plan (stage = conv(conv(x))):
  G setup     : G = DV^-1/2 Hs DE^-1 Hs^T DV^-1/2 computed on-device (tiny, fp32).
                G is symmetric. g = G @ 1 for the aggregated-bias term.
  phase A     : A_fm[d,(b,m)] = (G X_b)^T   -- AGG-B: activation-stationary
                matmuls (lhsT=X_b[80,128-dtile], rhs=G) -> RM->FM "free" transpose.
  phase B     : H_fm = relu(A_fm.T W1 + g (x) b1) -- weight-stationary matmuls
                accumulating over din tiles + a K=1 bias-row matmul; ACT relu
                copyback straight from PSUM (FM->FM).
  phase C     : per (dout-chunk, batch): Y = H_b^T W2 + b2 (activation-stationary,
                FM->RM), then Z = G Y (G-stationary), DMA out.
"""
import numpy as np

_CACHE = {}

B_PER_CORE = 16
NN = 80
R = B_PER_CORE * NN  # 1280
N_CORES = 8


def _build_program():
    import concourse.mybir as mybir
    import concourse.tile as tile
    from concourse import bacc
    from concourse.masks import make_identity

    dt = mybir.dt
    AF = mybir.ActivationFunctionType
    ALU = mybir.AluOpType
    f32r = dt.float32r
    f32 = dt.float32

    B = B_PER_CORE
    RCHUNKS = [(0, 512), (512, 512), (1024, 256)]
    BGROUPS = [(0, 6), (6, 6), (12, 4)]

    nc = bacc.Bacc("TRN2", target_bir_lowering=False, debug=False)

    x3_d = nc.dram_tensor("x3", [B, NN, 1024], f32r, kind="ExternalInput").ap()
    x4_d = nc.dram_tensor("x4", [B, NN, 2048], f32r, kind="ExternalInput").ap()
    H_d = nc.dram_tensor("H", [NN, NN], f32, kind="ExternalInput").ap()
    w31_d = nc.dram_tensor("w31", [1024, 1024], f32r, kind="ExternalInput").ap()
    w32_d = nc.dram_tensor("w32", [1024, 1024], f32r, kind="ExternalInput").ap()
    w41_d = nc.dram_tensor("w41", [2048, 2048], f32r, kind="ExternalInput").ap()
    w42_d = nc.dram_tensor("w42", [2048, 2048], f32r, kind="ExternalInput").ap()
    b31_d = nc.dram_tensor("b31", [1, 1024], f32r, kind="ExternalInput").ap()
    b32_d = nc.dram_tensor("b32", [1, 1024], f32r, kind="ExternalInput").ap()
    b41_d = nc.dram_tensor("b41", [1, 2048], f32r, kind="ExternalInput").ap()
    b42_d = nc.dram_tensor("b42", [1, 2048], f32r, kind="ExternalInput").ap()
    out_d = nc.dram_tensor("out", [B, NN, 3072], f32, kind="ExternalOutput").ap()

    with tile.TileContext(nc) as tc:
        with tc.tile_pool(name="const", bufs=1) as cpool:
            G_r = cpool.tile([NN, NN], f32r)
            grow_r = cpool.tile([1, R], f32r)
            ones80_r = cpool.tile([1, NN], f32r)

            # ---- G setup (tiny, fp32) ----
            with tc.tile_pool(name="gsetup", bufs=1) as gp, \
                 tc.tile_pool(name="gps", bufs=1, space="PSUM") as gpsum:
                ident = gp.tile([NN, NN], f32)
                make_identity(nc, ident[:])
                ones_col = gp.tile([NN, 1], f32)
                nc.vector.memset(ones_col[:], 1.0)
                Hsb = gp.tile([NN, NN], f32)
                nc.sync.dma_start(Hsb[:], H_d)
                Hs = gp.tile([NN, NN], f32)
                nc.scalar.activation(Hs[:], Hsb[:], AF.Sigmoid)
                dv = gp.tile([NN, 1], f32)
                nc.vector.tensor_reduce(dv[:], Hs[:], mybir.AxisListType.X, ALU.add)
                sq = gp.tile([NN, 1], f32)
                nc.scalar.sqrt(sq[:], dv[:])
                dv2 = gp.tile([NN, 1], f32)
                nc.vector.reciprocal(dv2[:], sq[:])
                Hp = gp.tile([NN, NN], f32)
                nc.scalar.mul(Hp[:], Hs[:], dv2[:])  # Hs * dv2[n]
                ps_de = gpsum.tile([NN, 1], f32)
                nc.tensor.matmul(ps_de[:], Hs[:], ones_col[:], start=True, stop=True)
                inv_de = gp.tile([NN, 1], f32)
                nc.vector.reciprocal(inv_de[:], ps_de[:])
                ps_hpt = gpsum.tile([NN, NN], f32)
                nc.tensor.matmul(ps_hpt[:], Hp[:], ident[:], start=True, stop=True)
                HpT = gp.tile([NN, NN], f32)
                nc.vector.tensor_copy(out=HpT[:], in_=ps_hpt[:])
                HpTs = gp.tile([NN, NN], f32)
                nc.scalar.mul(HpTs[:], ps_hpt[:], inv_de[:])  # HpT * inv_de[e]
                ps_G = gpsum.tile([NN, NN], f32)
                nc.tensor.matmul(ps_G[:], HpTs[:], HpT[:], start=True, stop=True)
                nc.vector.tensor_copy(out=G_r[:], in_=ps_G[:])
                G32 = gp.tile([NN, NN], f32)
                nc.scalar.copy(G32[:], ps_G[:])
                ps_g = gpsum.tile([NN, 1], f32)
                nc.tensor.matmul(ps_g[:], G32[:], ones_col[:], start=True, stop=True)
                g_col = gp.tile([NN, 1], f32)
                nc.vector.tensor_copy(out=g_col[:], in_=ps_g[:])
                ps_gr = gpsum.tile([1, NN], f32)
                nc.tensor.matmul(ps_gr[:], g_col[:], ident[:], start=True, stop=True)
                g_row = gp.tile([1, NN], f32)
                nc.vector.tensor_copy(out=g_row[:], in_=ps_gr[:])
                for b in range(B):
                    nc.vector.tensor_copy(out=grow_r[:, b * NN:(b + 1) * NN], in_=g_row[:])
                ones80_f = gp.tile([1, NN], f32)
                nc.vector.memset(ones80_f[:], 1.0)
                nc.vector.tensor_copy(out=ones80_r[:], in_=ones80_f[:])

            def build_stage(x_d, w1_d, b1_d, w2_d, b2_d, col_off, D):
                KT = D // 128
                DC = D // 512
                # non-LIFO pool lifetimes (queue alloc mode):
                #   biasp, afm | xp,psA (phase A) | hfm, wp,psB (phase B) |
                #   free afm | w2p,yz,psY,psZ (phase C)
                biasp_cm = tc.tile_pool(name=f"bias{D}", bufs=1)
                biasp = biasp_cm.__enter__()
                b1_s = biasp.tile([1, D], f32r)
                b2_s = biasp.tile([1, D], f32r)
                nc.sync.dma_start(b1_s[:], b1_d)
                nc.sync.dma_start(b2_s[:], b2_d)
                afm_cm = tc.tile_pool(name=f"afm{D}", bufs=1, side="right")
                afm_pool = afm_cm.__enter__()
                A_fm = afm_pool.tile([128, KT, R], f32r)
                # phase A: AGG-B (RM -> FM)
                with tc.tile_pool(name=f"xp{D}", bufs=2) as xpool, \
                     tc.tile_pool(name=f"psA{D}", bufs=2, space="PSUM") as psumA:
                    for (b0, blen) in BGROUPS:
                        xg = xpool.tile([NN, 6, D], f32r, tag="xg")
                        nc.sync.dma_start(
                            xg[:, :blen],
                            x_d[b0:b0 + blen].rearrange("b n d -> n b d"))
                        for kt in range(KT):
                            psA = psumA.tile([128, 6 * NN], f32)
                            for j in range(blen):
                                nc.tensor.matmul(
                                    psA[:, j * NN:(j + 1) * NN],
                                    xg[:, j, kt * 128:(kt + 1) * 128],
                                    G_r[:],
                                    start=True, stop=True)
                            nc.vector.tensor_copy(
                                out=A_fm[:, kt, b0 * NN:(b0 + blen) * NN],
                                in_=psA[:, :blen * NN])
                hfm_cm = tc.tile_pool(name=f"hfm{D}", bufs=1)
                hfm_pool = hfm_cm.__enter__()
                H_fm = hfm_pool.tile([128, KT, R], f32r)
                # phase B: MUL-A + bias + relu (FM -> FM)
                with tc.tile_pool(name=f"wp{D}", bufs=2) as wpool, \
                     tc.tile_pool(name=f"psB{D}", bufs=4, space="PSUM") as psumB:
                    for dto in range(KT):
                        w1t = wpool.tile([128, KT, 128], f32r, tag="w1t")
                        nc.sync.dma_start(
                            w1t[:],
                            w1_d[:, dto * 128:(dto + 1) * 128]
                            .rearrange("(kt p) m -> p kt m", p=128))
                        for (r0, rl) in RCHUNKS:
                            ps = psumB.tile([128, 512], f32)
                            for kt in range(KT):
                                nc.tensor.matmul(
                                    ps[:, :rl], w1t[:, kt],
                                    A_fm[:, kt, r0:r0 + rl],
                                    start=(kt == 0), stop=False)
                            nc.tensor.matmul(
                                ps[:, :rl],
                                b1_s[:, dto * 128:(dto + 1) * 128],
                                grow_r[:, r0:r0 + rl],
                                start=False, stop=True)
                            nc.scalar.activation(
                                H_fm[:, dto, r0:r0 + rl], ps[:, :rl], AF.Relu)
                afm_cm.__exit__(None, None, None)
                # phase C: MUL-B + bias (FM -> RM), AGG-A, DMA out
                with tc.tile_pool(name=f"w2p{D}", bufs=2) as w2pool, \
                     tc.tile_pool(name=f"yz{D}", bufs=3) as yzpool, \
                     tc.tile_pool(name=f"psY{D}", bufs=2, space="PSUM") as psumY, \
                     tc.tile_pool(name=f"psZ{D}", bufs=2, space="PSUM") as psumZ:
                    for dc in range(DC):
                        w2c = w2pool.tile([128, KT, 512], f32r, tag="w2c")
                        nc.sync.dma_start(
                            w2c[:],
                            w2_d[:, dc * 512:(dc + 1) * 512]
                            .rearrange("(kt p) n -> p kt n", p=128))
                        for b in range(B):
                            psy = psumY.tile([NN, 512], f32)
                            for kt in range(KT):
                                nc.tensor.matmul(
                                    psy[:], H_fm[:, kt, b * NN:(b + 1) * NN],
                                    w2c[:, kt], start=(kt == 0), stop=False)
                            nc.tensor.matmul(
                                psy[:], ones80_r[:],
                                b2_s[:, dc * 512:(dc + 1) * 512],
                                start=False, stop=True)
                            ysb = yzpool.tile([NN, 512], f32r, tag="y")
                            nc.vector.tensor_copy(out=ysb[:], in_=psy[:])
                            psz = psumZ.tile([NN, 512], f32)
                            nc.tensor.matmul(psz[:], G_r[:], ysb[:], start=True, stop=True)
                            zsb = yzpool.tile([NN, 512], f32, tag="z")
                            nc.scalar.copy(zsb[:], psz[:])
                            nc.sync.dma_start(
                                out_d[b, :, col_off + dc * 512:col_off + (dc + 1) * 512],
                                zsb[:])
                hfm_cm.__exit__(None, None, None)
                biasp_cm.__exit__(None, None, None)

            build_stage(x3_d, w31_d, b31_d, w32_d, b32_d, 0, 1024)
            build_stage(x4_d, w41_d, b41_d, w42_d, b42_d, 1024, 2048)

    nc.compile()
    return nc


def get_program():
    if "nc" not in _CACHE:
        _CACHE["nc"] = _build_program()
    return _CACHE["nc"]


def make_in_maps(inputs):
    x3 = np.ascontiguousarray(np.asarray(inputs["stage_3_input"], dtype=np.float32))
    x4 = np.ascontiguousarray(np.asarray(inputs["input_x"], dtype=np.float32))
    H = np.ascontiguousarray(np.asarray(inputs["H"], dtype=np.float32))
    ws = {k: np.ascontiguousarray(np.asarray(inputs[k], dtype=np.float32))
          for k in ("w31", "w32", "w41", "w42")}
    bs = {k: np.ascontiguousarray(np.asarray(inputs[k], dtype=np.float32).reshape(1, -1))
          for k in ("b31", "b32", "b41", "b42")}
    in_maps = []
    for c in range(N_CORES):
        sl = slice(c * B_PER_CORE, (c + 1) * B_PER_CORE)
        in_maps.append({
            "x3": x3[sl], "x4": x4[sl], "H": H,
            "w31": ws["w31"], "w32": ws["w32"],
            "w41": ws["w41"], "w42": ws["w42"],
            "b31": bs["b31"], "b32": bs["b32"],
            "b41": bs["b41"], "b42": bs["b42"],
        })
    return in_maps


def kernel(**inputs):
    from concourse.bass_utils import run_bass_kernel_spmd
    nc = get_program()
    in_maps = make_in_maps(inputs)
    res = run_bass_kernel_spmd(nc, in_maps, list(range(N_CORES)))
    out = np.concatenate([res.results[c]["out"] for c in range(N_CORES)], axis=0)
    return np.ascontiguousarray(out.astype(np.float32))
